# revision 12
# baseline (speedup 1.0000x reference)
"""AttnDecoderRNN step distributed across 8 TRN2 NeuronCores (Bass/Tile).

Strategy (tensor parallel, 8 cores):
- Every small GEMV in the recurrent cell is sharded on its OUTPUT dim; the
  full activation vector is re-materialized on every core with an AllGather.
- The embedding table is sharded on V (8192 rows/core, padded); each core
  dynamic-slice-gathers its local candidate row, an AllGather + dynamic row
  pick broadcasts the owner's row.
- The dominant cost, the [V,H] output projection (206 MB), is sharded on V:
  6400 cols/core (padded; pad bias = -1e30 so log-softmax ignores it).
  Weights stream through SBUF in [128,1600] tiles; fp32r matmuls; flash-style
  max/sumexp stats per 320-wide chunk; one tiny AllGather of (max, sumexp)
  implements the distributed logsumexp; each core writes its logit shard.
"""

import numpy as np

import concourse.bass as bass
import concourse.tile as tile
from concourse import bacc, mybir
from concourse import bass_utils

F32 = mybir.dt.float32
F32R = mybir.dt.float32r
I32 = mybir.dt.int32

H = 1024          # hidden size
V = 50257         # vocab
L = 512           # encoder length
NCORES = 8
ES = 8192         # emb shard rows (power of two; owner = x >> 13, xloc = x & 8191)
VPAD = ES * NCORES
VS = 6400         # out-projection shard width (V padded to 51200)
NBLK = 4          # out_W N-blocks per core
BLKW = VS // NBLK         # 1600
CHW = 400                 # psum chunk width (>=256 keeps fp32r at 1 cyc/row)
CPB = BLKW // CHW         # 4 chunks per block
NCH = NBLK * CPB          # 16 chunks
NEG = -1.0e30


def _build():
    nc = bacc.Bacc("TRN2", target_bir_lowering=False, debug=False,
                   enable_asserts=False, num_devices=NCORES)

    # ---- per-core external inputs ----
    x_d = nc.dram_tensor("x32", [1], I32, kind="ExternalInput")
    h0_d = nc.dram_tensor("h0", [H], F32, kind="ExternalInput")
    emb_d = nc.dram_tensor("emb_sh", [ES, H], F32, kind="ExternalInput")
    attnw_d = nc.dram_tensor("attnWT", [128, 16 * 64], F32, kind="ExternalInput")
    attnb_d = nc.dram_tensor("attnb", [64], F32, kind="ExternalInput")
    enc_d = nc.dram_tensor("encT", [128, 4 * 128], F32, kind="ExternalInput")
    combw_d = nc.dram_tensor("combWT", [128, 16 * 128], F32, kind="ExternalInput")
    combb_d = nc.dram_tensor("combb", [128], F32, kind="ExternalInput")
    wih_d = nc.dram_tensor("WihT", [128, 8 * 384], F32, kind="ExternalInput")
    whh_d = nc.dram_tensor("WhhT", [128, 8 * 384], F32, kind="ExternalInput")
    bih_d = nc.dram_tensor("bih", [384], F32, kind="ExternalInput")
    bhh_d = nc.dram_tensor("bhh", [384], F32, kind="ExternalInput")
    h0sl_d = nc.dram_tensor("h0sl", [128], F32, kind="ExternalInput")
    outw_d = nc.dram_tensor("outWT", [8, 128, VS], F32, kind="ExternalInput")
    outb_d = nc.dram_tensor("outb", [VS], F32, kind="ExternalInput")

    # ---- per-core external outputs ----
    outz_d = nc.dram_tensor("out_z", [VS], F32, kind="ExternalOutput")
    outa_d = nc.dram_tensor("out_attn", [L], F32, kind="ExternalOutput")

    with tile.TileContext(nc) as tc:
        with (
            tc.tile_pool(name="consts", bufs=1) as consts,
            tc.tile_pool(name="vecs", bufs=1) as vecs,
            tc.tile_pool(name="wpool", bufs=2) as wpool,
            tc.tile_pool(name="chunk", bufs=2) as chunk_pool,
            tc.tile_pool(name="ps_small", bufs=1, space="PSUM") as ps_small,
            tc.tile_pool(name="ps_big", bufs=CPB, space="PSUM") as ps_big,
            tc.tile_pool(name="ps_warm", bufs=1, space="PSUM") as ps_warm,
            tc.tile_pool(name="dram", bufs=1, space="DRAM") as dram,
        ):
            # ===== constant loads =====
            x_sb = consts.tile([1, 1], I32)
            nc.scalar.dma_start(x_sb[:], x_d.ap().rearrange("(a b) -> a b", a=1))

            attnw_sb = consts.tile([128, 16 * 64], F32R)
            nc.sync.dma_start(attnw_sb[:], attnw_d.ap().bitcast(F32R))
            attnb_sb = consts.tile([1, 64], F32)
            nc.sync.dma_start(attnb_sb[:], attnb_d.ap().rearrange("(a b) -> a b", a=1))
            enc_sb = consts.tile([128, 4 * 128], F32R)
            nc.sync.dma_start(enc_sb[:], enc_d.ap().bitcast(F32R))
            combw_sb = consts.tile([128, 16 * 128], F32R)
            nc.sync.dma_start(combw_sb[:], combw_d.ap().bitcast(F32R))
            combb_sb = consts.tile([1, 128], F32)
            nc.sync.dma_start(combb_sb[:], combb_d.ap().rearrange("(a b) -> a b", a=1))
            wih_sb = consts.tile([128, 8 * 384], F32R)
            nc.sync.dma_start(wih_sb[:], wih_d.ap().bitcast(F32R))
            whh_sb = consts.tile([128, 8 * 384], F32R)
            nc.sync.dma_start(whh_sb[:], whh_d.ap().bitcast(F32R))
            bih_sb = consts.tile([1, 384], F32)
            nc.sync.dma_start(bih_sb[:], bih_d.ap().rearrange("(a b) -> a b", a=1))
            bhh_sb = consts.tile([1, 384], F32)
            nc.sync.dma_start(bhh_sb[:], bhh_d.ap().rearrange("(a b) -> a b", a=1))
            h0sl_sb = consts.tile([1, 128], F32)
            nc.sync.dma_start(h0sl_sb[:], h0sl_d.ap().rearrange("(a b) -> a b", a=1))

            # h0 in the three layouts we need
            h08_sb = consts.tile([128, 8], F32R)  # p-major for GRU gh GEMV
            nc.sync.dma_start(
                h08_sb[:], h0_d.ap().rearrange("(p f) -> p f", p=128).bitcast(F32R))
            h0cat_sb = consts.tile([64, 16], F32R)  # rows 64..127 of concat layout
            nc.sync.dma_start(
                h0cat_sb[:], h0_d.ap().rearrange("(p f) -> p f", p=64).bitcast(F32R))
            ones_sb = consts.tile([1, 128], F32)
            nc.vector.memset(ones_sb[:], 1.0)

            # ===== collective bounce buffers (DRAM) =====
            ag1i = dram.tile([1, H], F32)
            ag1o = dram.tile([NCORES, H], F32)
            ag2i = dram.tile([1, 64], F32)
            ag2o = dram.tile([NCORES, 64], F32)
            ag3i = dram.tile([1, 128], F32)
            ag3o = dram.tile([NCORES, 128], F32)
            ag4i = dram.tile([1, 128], F32)
            ag4o = dram.tile([NCORES, 128], F32)
            ag5i = dram.tile([1, 128], F32)
            ag5o = dram.tile([NCORES, 128], F32)
            ag6i = dram.tile([1, 2], F32)
            ag6o = dram.tile([NCORES, 2], F32)
            awscr = dram.tile([1, L], F32)

            def allgather(i_t, o_t):
                nc.gpsimd.collective_compute(
                    "AllGather", mybir.AluOpType.bypass,
                    replica_groups=[list(range(NCORES))],
                    ins=[i_t.opt()], outs=[o_t.opt()],
                )

            # warm-up collective: absorbs the communicator-init barrier and
            # first-collective overhead concurrently with the weight prefetch
            ag0i = dram.tile([1, 2], F32)
            ag0o = dram.tile([NCORES, 2], F32)
            allgather(ag0i, ag0o)

            # ===== out_W streaming + PE warm-up (independent of the chain) =====
            wtiles = {}
            for b in range(NBLK):
                for t in range(8):
                    wt = wpool.tile([128, BLKW], F32R, tag=f"w{t}",
                                    bufs=3 if t < 6 else 2)
                    nc.sync.dma_start(
                        wt[:], outw_d.ap()[t, :, b * BLKW:(b + 1) * BLKW].bitcast(F32R))
                    wtiles[(b, t)] = wt
                    # dummy matmul keeps the PE HAM warm through the serial chain
                    wps = ps_warm.tile([1, 512], F32, tag="warm")
                    nc.tensor.matmul(out=wps[:], lhsT=wt[:, 0:1], rhs=wt[:, 0:512],
                                     start=True, stop=True)

            # ===== embedding gather =====
            xv = nc.values_load(x_sb[0:1, 0:1], min_val=0, max_val=V - 1,
                                skip_runtime_bounds_check=True)
            owner = xv >> 13
            xloc = xv & (ES - 1)
            emb_v = emb_d.ap().rearrange("v (p f) -> v p f", p=64)
            g64 = vecs.tile([64, 16], F32)
            nc.gpsimd.dma_start(g64[:], emb_v[bass.ds(xloc, 1), :, :])
            nc.scalar.dma_start(ag1i[0:1, :].rearrange("a (p f) -> (a p) f", p=64), g64[:])
            allgather(ag1i, ag1o)

            # pick owner's row into the two concat-layout tiles
            g1v = ag1o[:, :].rearrange("c (p f) -> c p f", p=64)
            xc1 = vecs.tile([128, 16], F32R)   # [emb; h0]
            nc.gpsimd.dma_start(xc1[0:64, :], g1v[bass.ds(owner, 1), :, :].bitcast(F32R))
            nc.vector.tensor_copy(out=xc1[64:128, :], in_=h0cat_sb[:])
            xc2 = vecs.tile([128, 16], F32R)   # [emb; attn_applied]
            nc.gpsimd.dma_start(xc2[0:64, :], g1v[bass.ds(owner, 1), :, :].bitcast(F32R))

            # ===== attention scores (shard of 64) + softmax =====
            ps_s = ps_small.tile([1, 64], F32, tag="ps_a")
            for t in range(16):
                nc.tensor.matmul(out=ps_s[:], lhsT=xc1[:, t:t + 1],
                                 rhs=attnw_sb[:, t * 64:(t + 1) * 64],
                                 start=(t == 0), stop=(t == 15))
            s_loc = vecs.tile([1, 64], F32)
            nc.vector.tensor_add(out=s_loc[:], in0=ps_s[:], in1=attnb_sb[:])
            nc.scalar.dma_start(ag2i[0:1, :], s_loc[:])
            allgather(ag2i, ag2o)

            s_full = vecs.tile([1, L], F32)
            nc.scalar.dma_start(
                s_full[:], ag2o[:, :].rearrange("c n -> (c n)").rearrange("(a n) -> a n", a=1))
            nms = vecs.tile([1, 1], F32)
            nc.vector.reduce_max(out=nms[:], in_=s_full[:],
                                 axis=mybir.AxisListType.X, negate=True)
            e_aw = vecs.tile([1, L], F32)
            ssum = vecs.tile([1, 1], F32)
            nc.scalar.activation(out=e_aw[:], in_=s_full[:],
                                 func=mybir.ActivationFunctionType.Exp,
                                 bias=nms[0:1, 0:1], scale=1.0, accum_out=ssum[0:1, 0:1])
            # redistribute the UNNORMALIZED exp to partition-major right away;
            # the 1/sum scale is folded into the attn_applied epilogue.
            nc.scalar.dma_start(awscr[0:1, :], e_aw[:])
            aw_dist = vecs.tile([128, 4], F32R)
            nc.scalar.dma_start(
                aw_dist[:], awscr[0:1, :].rearrange("a (p j) -> (a p) j", p=128).bitcast(F32R))
            rsum = vecs.tile([1, 1], F32)
            nc.vector.reciprocal(out=rsum[:], in_=ssum[:])
            # attention-weights output (off the critical path)
            aw_sb = vecs.tile([1, L], F32)
            nc.vector.tensor_scalar_mul(out=aw_sb[:], in0=e_aw[:], scalar1=rsum[0:1, 0:1])
            nc.gpsimd.dma_start(outa_d.ap().rearrange("(a n) -> a n", a=1), aw_sb[:])

            # ===== attn_applied shard (128 cols of H) =====
            ps_aa = ps_small.tile([1, 128], F32, tag="ps_a")
            for t in range(4):
                nc.tensor.matmul(out=ps_aa[:], lhsT=aw_dist[:, t:t + 1],
                                 rhs=enc_sb[:, t * 128:(t + 1) * 128],
                                 start=(t == 0), stop=(t == 3))
            aa_loc = vecs.tile([1, 128], F32)
            nc.vector.tensor_scalar_mul(out=aa_loc[:], in0=ps_aa[:], scalar1=rsum[0:1, 0:1])
            nc.scalar.dma_start(ag3i[0:1, :], aa_loc[:])
            allgather(ag3i, ag3o)
            nc.gpsimd.dma_start(
                xc2[64:128, :],
                ag3o[:, :].rearrange("c (q j) -> (c q) j", q=8).bitcast(F32R))

            # ===== combine + relu -> GRU input shard (128) =====
            ps_c = ps_small.tile([1, 128], F32, tag="ps_a")
            for t in range(16):
                nc.tensor.matmul(out=ps_c[:], lhsT=xc2[:, t:t + 1],
                                 rhs=combw_sb[:, t * 128:(t + 1) * 128],
                                 start=(t == 0), stop=(t == 15))
            gi_loc = vecs.tile([1, 128], F32)
            nc.vector.tensor_add(out=gi_loc[:], in0=ps_c[:], in1=combb_sb[:])
            nc.vector.tensor_scalar_max(out=gi_loc[:], in0=gi_loc[:], scalar1=0.0)
            nc.scalar.dma_start(ag4i[0:1, :], gi_loc[:])
            allgather(ag4i, ag4o)
            gru8 = vecs.tile([128, 8], F32R)
            nc.gpsimd.dma_start(
                gru8[:],
                ag4o[:, :].rearrange("c (p f) -> (c p) f", p=16).bitcast(F32R))

            # ===== GRU cell (shard: rows k*128..k*128+128 of h) =====
            ps_gx = ps_small.tile([1, 384], F32, tag="ps_gx")
            ps_gh = ps_small.tile([1, 384], F32, tag="ps_gh")
            for t in range(8):
                nc.tensor.matmul(out=ps_gx[:], lhsT=gru8[:, t:t + 1],
                                 rhs=wih_sb[:, t * 384:(t + 1) * 384],
                                 start=(t == 0), stop=(t == 7))
            for t in range(8):
                nc.tensor.matmul(out=ps_gh[:], lhsT=h08_sb[:, t:t + 1],
                                 rhs=whh_sb[:, t * 384:(t + 1) * 384],
                                 start=(t == 0), stop=(t == 7))
            gx = vecs.tile([1, 384], F32)
            gh = vecs.tile([1, 384], F32)
            nc.vector.tensor_add(out=gx[:], in0=ps_gx[:], in1=bih_sb[:])
            nc.vector.tensor_add(out=gh[:], in0=ps_gh[:], in1=bhh_sb[:])

            t_rz = vecs.tile([1, 256], F32)
            nc.vector.tensor_add(out=t_rz[:], in0=gx[0:1, 0:256], in1=gh[0:1, 0:256])
            rz_g = vecs.tile([1, 256], F32)
            nc.scalar.activation(out=rz_g[:], in_=t_rz[:],
                                 func=mybir.ActivationFunctionType.Sigmoid)
            t_n = vecs.tile([1, 128], F32)
            nc.vector.tensor_mul(out=t_n[:], in0=rz_g[0:1, 0:128], in1=gh[0:1, 256:384])
            nc.vector.tensor_add(out=t_n[:], in0=t_n[:], in1=gx[0:1, 256:384])
            n_g = vecs.tile([1, 128], F32)
            nc.scalar.activation(out=n_g[:], in_=t_n[:],
                                 func=mybir.ActivationFunctionType.Tanh)
            # h_new = n + z*(h0 - n)
            hn = vecs.tile([1, 128], F32)
            nc.vector.tensor_sub(out=hn[:], in0=h0sl_sb[:], in1=n_g[:])
            nc.vector.tensor_mul(out=hn[:], in0=hn[:], in1=rz_g[0:1, 128:256])
            nc.vector.tensor_add(out=hn[:], in0=hn[:], in1=n_g[:])
            nc.scalar.dma_start(ag5i[0:1, :], hn[:])
            allgather(ag5i, ag5o)
            hnew8 = vecs.tile([128, 8], F32R)
            nc.gpsimd.dma_start(
                hnew8[:],
                ag5o[:, :].rearrange("c (p f) -> (c p) f", p=16).bitcast(F32R))

            # ===== big matvec: z = h_new @ out_W_shard.T + out_b_shard =====
            zscr = dram.tile([1, VS], F32)
            stats_m = vecs.tile([1, NCH], F32)   # negated chunk maxes
            stats_s = vecs.tile([1, NCH], F32)   # chunk sumexp (rel. chunk max)
            for b in range(NBLK):
                pss = []
                for ci in range(CPB):
                    ps_z = ps_big.tile([1, CHW], F32, tag="ps_z")
                    pss.append(ps_z)
                for t in range(8):
                    wt = wtiles[(b, t)]
                    for ci in range(CPB):
                        nc.tensor.matmul(
                            out=pss[ci][:], lhsT=hnew8[:, t:t + 1],
                            rhs=wt[:, ci * CHW:(ci + 1) * CHW],
                            start=(t == 0), stop=(t == 7))
                for ci in range(CPB):
                    c = b * CPB + ci
                    c0 = c * CHW
                    bias_c = chunk_pool.tile([1, CHW], F32, tag="bias")
                    nc.gpsimd.dma_start(
                        bias_c[:],
                        outb_d.ap()[c0:c0 + CHW].rearrange("(a n) -> a n", a=1))
                    zc = chunk_pool.tile([1, CHW], F32, tag="z", bufs=3)
                    nc.vector.tensor_add(out=zc[:], in0=pss[ci][:], in1=bias_c[:])
                    nc.vector.reduce_max(out=stats_m[0:1, c:c + 1], in_=zc[:],
                                         axis=mybir.AxisListType.X, negate=True)
                    e_c = chunk_pool.tile([1, CHW], F32, tag="e")
                    nc.scalar.activation(out=e_c[:], in_=zc[:],
                                         func=mybir.ActivationFunctionType.Exp,
                                         bias=stats_m[0:1, c:c + 1], scale=1.0,
                                         accum_out=stats_s[0:1, c:c + 1])
                    nc.scalar.dma_start(zscr[0:1, c0:c0 + CHW], zc[:])

            # local flash-combine: m_k = max_c m_c ; s_k = sum_c s_c*exp(m_c-m_k)
            nm = vecs.tile([1, 1], F32)
            nc.vector.tensor_reduce(out=nm[:], in_=stats_m[:],
                                    axis=mybir.AxisListType.X, op=mybir.AluOpType.min)
            wts = vecs.tile([1, NCH], F32)
            nc.scalar.activation(out=wts[:], in_=stats_m[:],
                                 func=mybir.ActivationFunctionType.Exp,
                                 bias=nm[0:1, 0:1], scale=-1.0)
            nc.vector.tensor_mul(out=wts[:], in0=wts[:], in1=stats_s[:])
            stats2 = vecs.tile([1, 2], F32)
            nc.vector.tensor_scalar_mul(out=stats2[0:1, 0:1], in0=nm[:], scalar1=-1.0)
            nc.vector.reduce_sum(out=stats2[0:1, 1:2], in_=wts[:],
                                 axis=mybir.AxisListType.X)
            nc.scalar.dma_start(ag6i[0:1, :], stats2[:])
            allgather(ag6i, ag6o)

            # global logsumexp from the 8 (m_k, s_k) pairs
            stat16 = vecs.tile([1, 16], F32)
            nc.scalar.dma_start(
                stat16[:], ag6o[:, :].rearrange("c s -> (c s)").rearrange("(a n) -> a n", a=1))
            sview = stat16[0:1, :].rearrange("a (c s) -> a c s", s=2)
            nM = vecs.tile([1, 1], F32)
            nc.vector.reduce_max(out=nM[:], in_=sview[:, :, 0:1],
                                 axis=mybir.AxisListType.XY, negate=True)
            w2 = vecs.tile([1, 8], F32)
            nc.scalar.activation(out=w2[:].rearrange("a (c s) -> a c s", s=1),
                                 in_=sview[:, :, 0:1],
                                 func=mybir.ActivationFunctionType.Exp,
                                 bias=nM[0:1, 0:1], scale=1.0)
            sw = vecs.tile([1, 8], F32)
            nc.vector.tensor_tensor(out=sw[:].rearrange("a (c s) -> a c s", s=1),
                                    in0=w2[:].rearrange("a (c s) -> a c s", s=1),
                                    in1=sview[:, :, 1:2], op=mybir.AluOpType.mult)
            S_g = vecs.tile([1, 1], F32)
            nc.vector.reduce_sum(out=S_g[:], in_=sw[:], axis=mybir.AxisListType.X)
            lnS = vecs.tile([1, 1], F32)
            nc.scalar.activation(out=lnS[:], in_=S_g[:],
                                 func=mybir.ActivationFunctionType.Ln)
            neg_lse = vecs.tile([1, 1], F32)
            nc.vector.tensor_sub(out=neg_lse[:], in0=nM[:], in1=lnS[:])

            # logp shard = z - lse, computed across all 128 partitions:
            # broadcast -lse via a K=1 ones-matmul, subtract, write out.
            zt = vecs.tile([128, VS // 128], F32)
            nc.scalar.dma_start(
                zt[:], zscr[0:1, :].rearrange("a (p j) -> (a p) j", p=128))
            bps = ps_small.tile([128, 1], F32, tag="ps_gx")
            nc.tensor.matmul(out=bps[:], lhsT=ones_sb[0:1, 0:128],
                             rhs=neg_lse[0:1, 0:1], start=True, stop=True)
            nc.vector.tensor_scalar_add(out=zt[:], in0=zt[:], scalar1=bps[:, 0:1])
            nc.scalar.dma_start(outz_d.ap().rearrange("(p j) -> p j", p=128), zt[:])

    nc.compile()
    return nc


_NC_CACHE = None


def _get_nc():
    global _NC_CACHE
    if _NC_CACHE is None:
        _NC_CACHE = _build()
    return _NC_CACHE


def _prep_in_maps(x, encoder_outputs, hidden, emb, attn_W, attn_b, comb_W, comb_b,
                  W_ih, W_hh, b_ih, b_hh, out_W, out_b):
    f32 = np.float32
    x32 = np.asarray(x, np.int64).astype(np.int32).reshape(1)
    h0 = np.ascontiguousarray(np.asarray(hidden, f32).reshape(H))
    emb = np.asarray(emb, f32)
    attn_W = np.asarray(attn_W, f32)
    attn_b = np.asarray(attn_b, f32)
    comb_W = np.asarray(comb_W, f32)
    comb_b = np.asarray(comb_b, f32)
    W_ih = np.asarray(W_ih, f32)
    W_hh = np.asarray(W_hh, f32)
    b_ih = np.asarray(b_ih, f32)
    b_hh = np.asarray(b_hh, f32)
    out_W = np.asarray(out_W, f32)
    out_b = np.asarray(out_b, f32)
    enc = np.asarray(encoder_outputs, f32)

    emb_pad = np.zeros((VPAD, H), f32)
    emb_pad[:V] = emb
    attnWT = attn_W.T.copy()       # [2H, L]
    combWT = comb_W.T.copy()       # [2H, H]
    WihT = W_ih.T.copy()           # [H, 3H]
    WhhT = W_hh.T.copy()           # [H, 3H]
    outW_pad = np.zeros((VS * NCORES, H), f32)
    outW_pad[:V] = out_W
    outb_pad = np.full((VS * NCORES,), NEG, f32)
    outb_pad[:V] = out_b

    in_maps = []
    for k in range(NCORES):
        sl64 = slice(64 * k, 64 * (k + 1))
        sl128 = slice(128 * k, 128 * (k + 1))
        gsel = np.r_[128 * k:128 * (k + 1),
                     H + 128 * k:H + 128 * (k + 1),
                     2 * H + 128 * k:2 * H + 128 * (k + 1)]
        outWT_k = outW_pad[VS * k:VS * (k + 1)].T  # [H, VS]
        m = {
            "x32": x32,
            "h0": h0,
            "emb_sh": np.ascontiguousarray(emb_pad[ES * k:ES * (k + 1)]),
            "attnWT": np.ascontiguousarray(
                attnWT[:, sl64].reshape(128, 16, 64).reshape(128, 16 * 64)),
            "attnb": np.ascontiguousarray(attn_b[sl64]),
            "encT": np.ascontiguousarray(
                enc[:, sl128].reshape(128, 4, 128).reshape(128, 4 * 128)),
            "combWT": np.ascontiguousarray(
                combWT[:, sl128].reshape(128, 16, 128).reshape(128, 16 * 128)),
            "combb": np.ascontiguousarray(comb_b[sl128]),
            "WihT": np.ascontiguousarray(
                WihT[:, gsel].reshape(128, 8, 384).reshape(128, 8 * 384)),
            "WhhT": np.ascontiguousarray(
                WhhT[:, gsel].reshape(128, 8, 384).reshape(128, 8 * 384)),
            "bih": np.ascontiguousarray(b_ih[gsel]),
            "bhh": np.ascontiguousarray(b_hh[gsel]),
            "h0sl": np.ascontiguousarray(h0[sl128]),
            "outWT": np.ascontiguousarray(
                outWT_k.reshape(128, 8, VS).transpose(1, 0, 2)),
            "outb": np.ascontiguousarray(outb_pad[VS * k:VS * (k + 1)]),
        }
        in_maps.append(m)
    return in_maps


def kernel(**inputs):
    nc = _get_nc()
    in_maps = _prep_in_maps(**inputs)
    res = bass_utils.run_bass_kernel_spmd(nc, in_maps, core_ids=list(range(NCORES)))
    logp = np.concatenate([res.results[k]["out_z"] for k in range(NCORES)])[:V]
    attn = res.results[0]["out_attn"]
    return logp.reshape(1, V).astype(np.float32), attn.reshape(1, L).astype(np.float32)


# revision 13
# speedup vs baseline: 1.2774x; 1.2774x over previous
"""AttnDecoderRNN step distributed across 8 TRN2 NeuronCores (Bass/Tile).

Strategy (tensor parallel, 8 cores):
- Every small GEMV in the recurrent cell is sharded on its OUTPUT dim; the
  full activation vector is re-materialized on every core with an AllGather.
- The embedding table is sharded on V (8192 rows/core, padded); each core
  dynamic-slice-gathers its local candidate row, an AllGather + dynamic row
  pick broadcasts the owner's row.
- The dominant cost, the [V,H] output projection (206 MB), is sharded on V:
  6400 cols/core (padded; pad bias = -1e30 so log-softmax ignores it).
  Weights stream through SBUF in [128,1600] tiles; fp32r matmuls; flash-style
  max/sumexp stats per 320-wide chunk; one tiny AllGather of (max, sumexp)
  implements the distributed logsumexp; each core writes its logit shard.
"""

import ml_dtypes
import numpy as np

import concourse.bass as bass
import concourse.tile as tile
from concourse import bacc, mybir
from concourse import bass_utils

F32 = mybir.dt.float32
F32R = mybir.dt.float32r
BF16 = mybir.dt.bfloat16
I32 = mybir.dt.int32

H = 1024          # hidden size
V = 50257         # vocab
L = 512           # encoder length
NCORES = 8
ES = 8192         # emb shard rows (power of two; owner = x >> 13, xloc = x & 8191)
VPAD = ES * NCORES
VS = 6400         # out-projection shard width (V padded to 51200)
NBLK = 4          # out_W N-blocks per core
BLKW = VS // NBLK         # 1600
CHW = 400                 # psum chunk width (>=256 keeps fp32r at 1 cyc/row)
CPB = BLKW // CHW         # 4 chunks per block
NCH = NBLK * CPB          # 16 chunks
NEG = -1.0e30


def _build():
    nc = bacc.Bacc("TRN2", target_bir_lowering=False, debug=False,
                   enable_asserts=False, num_devices=NCORES)

    # ---- per-core external inputs ----
    x_d = nc.dram_tensor("x32", [1], I32, kind="ExternalInput")
    h0_d = nc.dram_tensor("h0", [H], F32, kind="ExternalInput")
    emb_d = nc.dram_tensor("emb_sh", [ES, H], F32, kind="ExternalInput")
    attnw_d = nc.dram_tensor("attnWT", [128, 16 * 64], F32, kind="ExternalInput")
    attnb_d = nc.dram_tensor("attnb", [64], F32, kind="ExternalInput")
    enc_d = nc.dram_tensor("encT", [128, 4 * 128], F32, kind="ExternalInput")
    combw_d = nc.dram_tensor("combWT", [128, 16 * 128], F32, kind="ExternalInput")
    combb_d = nc.dram_tensor("combb", [128], F32, kind="ExternalInput")
    wih_d = nc.dram_tensor("WihT", [128, 8 * 384], F32, kind="ExternalInput")
    whh_d = nc.dram_tensor("WhhT", [128, 8 * 384], F32, kind="ExternalInput")
    bih_d = nc.dram_tensor("bih", [384], F32, kind="ExternalInput")
    bhh_d = nc.dram_tensor("bhh", [384], F32, kind="ExternalInput")
    h0sl_d = nc.dram_tensor("h0sl", [128], F32, kind="ExternalInput")
    outw_d = nc.dram_tensor("outWT", [8, 128, VS], BF16, kind="ExternalInput")
    outb_d = nc.dram_tensor("outb", [VS], F32, kind="ExternalInput")

    # ---- per-core external outputs ----
    outz_d = nc.dram_tensor("out_z", [VS], F32, kind="ExternalOutput")
    outa_d = nc.dram_tensor("out_attn", [L], F32, kind="ExternalOutput")

    with tile.TileContext(nc) as tc:
        with (
            tc.tile_pool(name="consts", bufs=1) as consts,
            tc.tile_pool(name="vecs", bufs=1) as vecs,
            tc.tile_pool(name="wpool", bufs=2) as wpool,
            tc.tile_pool(name="chunk", bufs=2) as chunk_pool,
            tc.tile_pool(name="ps_small", bufs=1, space="PSUM") as ps_small,
            tc.tile_pool(name="ps_big", bufs=CPB, space="PSUM") as ps_big,
            tc.tile_pool(name="ps_warm", bufs=1, space="PSUM") as ps_warm,
            tc.tile_pool(name="dram", bufs=1, space="DRAM") as dram,
        ):
            # ===== constant loads =====
            x_sb = consts.tile([1, 1], I32)
            nc.scalar.dma_start(x_sb[:], x_d.ap().rearrange("(a b) -> a b", a=1))

            attnw_sb = consts.tile([128, 16 * 64], F32R)
            nc.sync.dma_start(attnw_sb[:], attnw_d.ap().bitcast(F32R))
            attnb_sb = consts.tile([1, 64], F32)
            nc.sync.dma_start(attnb_sb[:], attnb_d.ap().rearrange("(a b) -> a b", a=1))
            enc_sb = consts.tile([128, 4 * 128], F32R)
            nc.sync.dma_start(enc_sb[:], enc_d.ap().bitcast(F32R))
            combw_sb = consts.tile([128, 16 * 128], F32R)
            nc.sync.dma_start(combw_sb[:], combw_d.ap().bitcast(F32R))
            combb_sb = consts.tile([1, 128], F32)
            nc.sync.dma_start(combb_sb[:], combb_d.ap().rearrange("(a b) -> a b", a=1))
            wih_sb = consts.tile([128, 8 * 384], F32R)
            nc.sync.dma_start(wih_sb[:], wih_d.ap().bitcast(F32R))
            whh_sb = consts.tile([128, 8 * 384], F32R)
            nc.sync.dma_start(whh_sb[:], whh_d.ap().bitcast(F32R))
            bih_sb = consts.tile([1, 384], F32)
            nc.sync.dma_start(bih_sb[:], bih_d.ap().rearrange("(a b) -> a b", a=1))
            bhh_sb = consts.tile([1, 384], F32)
            nc.sync.dma_start(bhh_sb[:], bhh_d.ap().rearrange("(a b) -> a b", a=1))
            h0sl_sb = consts.tile([1, 128], F32)
            nc.sync.dma_start(h0sl_sb[:], h0sl_d.ap().rearrange("(a b) -> a b", a=1))

            # h0 in the three layouts we need
            h08_sb = consts.tile([128, 8], F32R)  # p-major for GRU gh GEMV
            nc.sync.dma_start(
                h08_sb[:], h0_d.ap().rearrange("(p f) -> p f", p=128).bitcast(F32R))
            h0cat_sb = consts.tile([64, 16], F32R)  # rows 64..127 of concat layout
            nc.sync.dma_start(
                h0cat_sb[:], h0_d.ap().rearrange("(p f) -> p f", p=64).bitcast(F32R))
            ones_sb = consts.tile([1, 128], F32)
            nc.vector.memset(ones_sb[:], 1.0)

            # ===== collective bounce buffers (DRAM) =====
            ag1i = dram.tile([1, H], F32)
            ag1o = dram.tile([NCORES, H], F32)
            ag2i = dram.tile([1, 64], F32)
            ag2o = dram.tile([NCORES, 64], F32)
            ag3i = dram.tile([1, 128], F32)
            ag3o = dram.tile([NCORES, 128], F32)
            ag4i = dram.tile([1, 128], F32)
            ag4o = dram.tile([NCORES, 128], F32)
            ag5i = dram.tile([1, 128], F32)
            ag5o = dram.tile([NCORES, 128], F32)
            ag6i = dram.tile([1, 2], F32)
            ag6o = dram.tile([NCORES, 2], F32)
            awscr = dram.tile([1, L], F32)

            def allgather(i_t, o_t):
                nc.gpsimd.collective_compute(
                    "AllGather", mybir.AluOpType.bypass,
                    replica_groups=[list(range(NCORES))],
                    ins=[i_t.opt()], outs=[o_t.opt()],
                )

            # warm-up collective: absorbs the communicator-init barrier and
            # first-collective overhead concurrently with the weight prefetch
            ag0i = dram.tile([1, 2], F32)
            ag0o = dram.tile([NCORES, 2], F32)
            allgather(ag0i, ag0o)

            # ===== out_W streaming + PE warm-up (independent of the chain) =====
            wtiles = {}
            for b in range(NBLK):
                for t in range(8):
                    wt = wpool.tile([128, BLKW], BF16, tag=f"w{t}", bufs=NBLK)
                    nc.sync.dma_start(
                        wt[:], outw_d.ap()[t, :, b * BLKW:(b + 1) * BLKW])
                    wtiles[(b, t)] = wt
                    # dummy matmul keeps the PE HAM warm through the serial chain
                    wps = ps_warm.tile([1, 512], F32, tag="warm")
                    nc.tensor.matmul(out=wps[:], lhsT=wt[:, 0:1], rhs=wt[:, 0:512],
                                     start=True, stop=True)

            # ===== embedding gather =====
            xv = nc.values_load(x_sb[0:1, 0:1], min_val=0, max_val=V - 1,
                                skip_runtime_bounds_check=True)
            owner = xv >> 13
            xloc = xv & (ES - 1)
            emb_v = emb_d.ap().rearrange("v (p f) -> v p f", p=64)
            g64 = vecs.tile([64, 16], F32)
            nc.gpsimd.dma_start(g64[:], emb_v[bass.ds(xloc, 1), :, :])
            nc.scalar.dma_start(ag1i[0:1, :].rearrange("a (p f) -> (a p) f", p=64), g64[:])
            allgather(ag1i, ag1o)

            # pick owner's row into the two concat-layout tiles
            g1v = ag1o[:, :].rearrange("c (p f) -> c p f", p=64)
            xc1 = vecs.tile([128, 16], F32R)   # [emb; h0]
            nc.gpsimd.dma_start(xc1[0:64, :], g1v[bass.ds(owner, 1), :, :].bitcast(F32R))
            nc.vector.tensor_copy(out=xc1[64:128, :], in_=h0cat_sb[:])
            xc2 = vecs.tile([128, 16], F32R)   # [emb; attn_applied]
            nc.gpsimd.dma_start(xc2[0:64, :], g1v[bass.ds(owner, 1), :, :].bitcast(F32R))

            # ===== attention scores (shard of 64) + softmax =====
            ps_s = ps_small.tile([1, 64], F32, tag="ps_a")
            for t in range(16):
                nc.tensor.matmul(out=ps_s[:], lhsT=xc1[:, t:t + 1],
                                 rhs=attnw_sb[:, t * 64:(t + 1) * 64],
                                 start=(t == 0), stop=(t == 15))
            s_loc = vecs.tile([1, 64], F32)
            nc.vector.tensor_add(out=s_loc[:], in0=ps_s[:], in1=attnb_sb[:])
            nc.scalar.dma_start(ag2i[0:1, :], s_loc[:])
            allgather(ag2i, ag2o)

            s_full = vecs.tile([1, L], F32)
            nc.scalar.dma_start(
                s_full[:], ag2o[:, :].rearrange("c n -> (c n)").rearrange("(a n) -> a n", a=1))
            nms = vecs.tile([1, 1], F32)
            nc.vector.reduce_max(out=nms[:], in_=s_full[:],
                                 axis=mybir.AxisListType.X, negate=True)
            e_aw = vecs.tile([1, L], F32)
            ssum = vecs.tile([1, 1], F32)
            nc.scalar.activation(out=e_aw[:], in_=s_full[:],
                                 func=mybir.ActivationFunctionType.Exp,
                                 bias=nms[0:1, 0:1], scale=1.0, accum_out=ssum[0:1, 0:1])
            # redistribute the UNNORMALIZED exp to partition-major right away;
            # the 1/sum scale is folded into the attn_applied epilogue.
            nc.scalar.dma_start(awscr[0:1, :], e_aw[:])
            aw_dist = vecs.tile([128, 4], F32R)
            nc.scalar.dma_start(
                aw_dist[:], awscr[0:1, :].rearrange("a (p j) -> (a p) j", p=128).bitcast(F32R))
            rsum = vecs.tile([1, 1], F32)
            nc.vector.reciprocal(out=rsum[:], in_=ssum[:])
            # attention-weights output (off the critical path)
            aw_sb = vecs.tile([1, L], F32)
            nc.vector.tensor_scalar_mul(out=aw_sb[:], in0=e_aw[:], scalar1=rsum[0:1, 0:1])
            nc.gpsimd.dma_start(outa_d.ap().rearrange("(a n) -> a n", a=1), aw_sb[:])

            # ===== attn_applied shard (128 cols of H) =====
            ps_aa = ps_small.tile([1, 128], F32, tag="ps_a")
            for t in range(4):
                nc.tensor.matmul(out=ps_aa[:], lhsT=aw_dist[:, t:t + 1],
                                 rhs=enc_sb[:, t * 128:(t + 1) * 128],
                                 start=(t == 0), stop=(t == 3))
            aa_loc = vecs.tile([1, 128], F32)
            nc.vector.tensor_scalar_mul(out=aa_loc[:], in0=ps_aa[:], scalar1=rsum[0:1, 0:1])
            nc.scalar.dma_start(ag3i[0:1, :], aa_loc[:])
            allgather(ag3i, ag3o)
            nc.gpsimd.dma_start(
                xc2[64:128, :],
                ag3o[:, :].rearrange("c (q j) -> (c q) j", q=8).bitcast(F32R))

            # ===== combine + relu -> GRU input shard (128) =====
            ps_c = ps_small.tile([1, 128], F32, tag="ps_a")
            for t in range(16):
                nc.tensor.matmul(out=ps_c[:], lhsT=xc2[:, t:t + 1],
                                 rhs=combw_sb[:, t * 128:(t + 1) * 128],
                                 start=(t == 0), stop=(t == 15))
            gi_loc = vecs.tile([1, 128], F32)
            nc.vector.tensor_add(out=gi_loc[:], in0=ps_c[:], in1=combb_sb[:])
            nc.vector.tensor_scalar_max(out=gi_loc[:], in0=gi_loc[:], scalar1=0.0)
            nc.scalar.dma_start(ag4i[0:1, :], gi_loc[:])
            allgather(ag4i, ag4o)
            gru8 = vecs.tile([128, 8], F32R)
            nc.gpsimd.dma_start(
                gru8[:],
                ag4o[:, :].rearrange("c (p f) -> (c p) f", p=16).bitcast(F32R))

            # ===== GRU cell (shard: rows k*128..k*128+128 of h) =====
            ps_gx = ps_small.tile([1, 384], F32, tag="ps_gx")
            ps_gh = ps_small.tile([1, 384], F32, tag="ps_gh")
            for t in range(8):
                nc.tensor.matmul(out=ps_gx[:], lhsT=gru8[:, t:t + 1],
                                 rhs=wih_sb[:, t * 384:(t + 1) * 384],
                                 start=(t == 0), stop=(t == 7))
            for t in range(8):
                nc.tensor.matmul(out=ps_gh[:], lhsT=h08_sb[:, t:t + 1],
                                 rhs=whh_sb[:, t * 384:(t + 1) * 384],
                                 start=(t == 0), stop=(t == 7))
            gx = vecs.tile([1, 384], F32)
            gh = vecs.tile([1, 384], F32)
            nc.vector.tensor_add(out=gx[:], in0=ps_gx[:], in1=bih_sb[:])
            nc.vector.tensor_add(out=gh[:], in0=ps_gh[:], in1=bhh_sb[:])

            t_rz = vecs.tile([1, 256], F32)
            nc.vector.tensor_add(out=t_rz[:], in0=gx[0:1, 0:256], in1=gh[0:1, 0:256])
            rz_g = vecs.tile([1, 256], F32)
            nc.scalar.activation(out=rz_g[:], in_=t_rz[:],
                                 func=mybir.ActivationFunctionType.Sigmoid)
            t_n = vecs.tile([1, 128], F32)
            nc.vector.tensor_mul(out=t_n[:], in0=rz_g[0:1, 0:128], in1=gh[0:1, 256:384])
            nc.vector.tensor_add(out=t_n[:], in0=t_n[:], in1=gx[0:1, 256:384])
            n_g = vecs.tile([1, 128], F32)
            nc.scalar.activation(out=n_g[:], in_=t_n[:],
                                 func=mybir.ActivationFunctionType.Tanh)
            # h_new = n + z*(h0 - n)
            hn = vecs.tile([1, 128], F32)
            nc.vector.tensor_sub(out=hn[:], in0=h0sl_sb[:], in1=n_g[:])
            nc.vector.tensor_mul(out=hn[:], in0=hn[:], in1=rz_g[0:1, 128:256])
            nc.vector.tensor_add(out=hn[:], in0=hn[:], in1=n_g[:])
            nc.scalar.dma_start(ag5i[0:1, :], hn[:])
            allgather(ag5i, ag5o)
            hnew8 = vecs.tile([128, 8], BF16)
            nc.gpsimd.dma_start(
                hnew8[:],
                ag5o[:, :].rearrange("c (p f) -> (c p) f", p=16))

            # ===== big matvec: z = h_new @ out_W_shard.T + out_b_shard =====
            zscr = dram.tile([1, VS], F32)
            stats_m = vecs.tile([1, NCH], F32)   # negated chunk maxes
            stats_s = vecs.tile([1, NCH], F32)   # chunk sumexp (rel. chunk max)
            for b in range(NBLK):
                pss = []
                for ci in range(CPB):
                    ps_z = ps_big.tile([1, CHW], F32, tag="ps_z")
                    pss.append(ps_z)
                for t in range(8):
                    wt = wtiles[(b, t)]
                    for ci in range(CPB):
                        nc.tensor.matmul(
                            out=pss[ci][:], lhsT=hnew8[:, t:t + 1],
                            rhs=wt[:, ci * CHW:(ci + 1) * CHW],
                            start=(t == 0), stop=(t == 7))
                for ci in range(CPB):
                    c = b * CPB + ci
                    c0 = c * CHW
                    bias_c = chunk_pool.tile([1, CHW], F32, tag="bias")
                    nc.gpsimd.dma_start(
                        bias_c[:],
                        outb_d.ap()[c0:c0 + CHW].rearrange("(a n) -> a n", a=1))
                    zc = chunk_pool.tile([1, CHW], F32, tag="z", bufs=3)
                    nc.vector.tensor_add(out=zc[:], in0=pss[ci][:], in1=bias_c[:])
                    nc.vector.reduce_max(out=stats_m[0:1, c:c + 1], in_=zc[:],
                                         axis=mybir.AxisListType.X, negate=True)
                    e_c = chunk_pool.tile([1, CHW], F32, tag="e")
                    nc.scalar.activation(out=e_c[:], in_=zc[:],
                                         func=mybir.ActivationFunctionType.Exp,
                                         bias=stats_m[0:1, c:c + 1], scale=1.0,
                                         accum_out=stats_s[0:1, c:c + 1])
                    nc.scalar.dma_start(zscr[0:1, c0:c0 + CHW], zc[:])

            # local flash-combine: m_k = max_c m_c ; s_k = sum_c s_c*exp(m_c-m_k)
            nm = vecs.tile([1, 1], F32)
            nc.vector.tensor_reduce(out=nm[:], in_=stats_m[:],
                                    axis=mybir.AxisListType.X, op=mybir.AluOpType.min)
            wts = vecs.tile([1, NCH], F32)
            nc.scalar.activation(out=wts[:], in_=stats_m[:],
                                 func=mybir.ActivationFunctionType.Exp,
                                 bias=nm[0:1, 0:1], scale=-1.0)
            nc.vector.tensor_mul(out=wts[:], in0=wts[:], in1=stats_s[:])
            stats2 = vecs.tile([1, 2], F32)
            nc.vector.tensor_scalar_mul(out=stats2[0:1, 0:1], in0=nm[:], scalar1=-1.0)
            nc.vector.reduce_sum(out=stats2[0:1, 1:2], in_=wts[:],
                                 axis=mybir.AxisListType.X)
            nc.scalar.dma_start(ag6i[0:1, :], stats2[:])
            allgather(ag6i, ag6o)

            # global logsumexp from the 8 (m_k, s_k) pairs
            stat16 = vecs.tile([1, 16], F32)
            nc.scalar.dma_start(
                stat16[:], ag6o[:, :].rearrange("c s -> (c s)").rearrange("(a n) -> a n", a=1))
            sview = stat16[0:1, :].rearrange("a (c s) -> a c s", s=2)
            nM = vecs.tile([1, 1], F32)
            nc.vector.reduce_max(out=nM[:], in_=sview[:, :, 0:1],
                                 axis=mybir.AxisListType.XY, negate=True)
            w2 = vecs.tile([1, 8], F32)
            nc.scalar.activation(out=w2[:].rearrange("a (c s) -> a c s", s=1),
                                 in_=sview[:, :, 0:1],
                                 func=mybir.ActivationFunctionType.Exp,
                                 bias=nM[0:1, 0:1], scale=1.0)
            sw = vecs.tile([1, 8], F32)
            nc.vector.tensor_tensor(out=sw[:].rearrange("a (c s) -> a c s", s=1),
                                    in0=w2[:].rearrange("a (c s) -> a c s", s=1),
                                    in1=sview[:, :, 1:2], op=mybir.AluOpType.mult)
            S_g = vecs.tile([1, 1], F32)
            nc.vector.reduce_sum(out=S_g[:], in_=sw[:], axis=mybir.AxisListType.X)
            lnS = vecs.tile([1, 1], F32)
            nc.scalar.activation(out=lnS[:], in_=S_g[:],
                                 func=mybir.ActivationFunctionType.Ln)
            neg_lse = vecs.tile([1, 1], F32)
            nc.vector.tensor_sub(out=neg_lse[:], in0=nM[:], in1=lnS[:])

            # logp shard = z - lse, computed across all 128 partitions:
            # broadcast -lse via a K=1 ones-matmul, subtract, write out.
            zt = vecs.tile([128, VS // 128], F32)
            nc.scalar.dma_start(
                zt[:], zscr[0:1, :].rearrange("a (p j) -> (a p) j", p=128))
            bps = ps_small.tile([128, 1], F32, tag="ps_gx")
            nc.tensor.matmul(out=bps[:], lhsT=ones_sb[0:1, 0:128],
                             rhs=neg_lse[0:1, 0:1], start=True, stop=True)
            nc.vector.tensor_scalar_add(out=zt[:], in0=zt[:], scalar1=bps[:, 0:1])
            nc.scalar.dma_start(outz_d.ap().rearrange("(p j) -> p j", p=128), zt[:])

    nc.compile()
    return nc


_NC_CACHE = None


def _get_nc():
    global _NC_CACHE
    if _NC_CACHE is None:
        _NC_CACHE = _build()
    return _NC_CACHE


def _prep_in_maps(x, encoder_outputs, hidden, emb, attn_W, attn_b, comb_W, comb_b,
                  W_ih, W_hh, b_ih, b_hh, out_W, out_b):
    f32 = np.float32
    x32 = np.asarray(x, np.int64).astype(np.int32).reshape(1)
    h0 = np.ascontiguousarray(np.asarray(hidden, f32).reshape(H))
    emb = np.asarray(emb, f32)
    attn_W = np.asarray(attn_W, f32)
    attn_b = np.asarray(attn_b, f32)
    comb_W = np.asarray(comb_W, f32)
    comb_b = np.asarray(comb_b, f32)
    W_ih = np.asarray(W_ih, f32)
    W_hh = np.asarray(W_hh, f32)
    b_ih = np.asarray(b_ih, f32)
    b_hh = np.asarray(b_hh, f32)
    out_W = np.asarray(out_W, f32)
    out_b = np.asarray(out_b, f32)
    enc = np.asarray(encoder_outputs, f32)

    emb_pad = np.zeros((VPAD, H), f32)
    emb_pad[:V] = emb
    attnWT = attn_W.T.copy()       # [2H, L]
    combWT = comb_W.T.copy()       # [2H, H]
    WihT = W_ih.T.copy()           # [H, 3H]
    WhhT = W_hh.T.copy()           # [H, 3H]
    outW_pad = np.zeros((VS * NCORES, H), f32)
    outW_pad[:V] = out_W
    outb_pad = np.full((VS * NCORES,), NEG, f32)
    outb_pad[:V] = out_b

    in_maps = []
    for k in range(NCORES):
        sl64 = slice(64 * k, 64 * (k + 1))
        sl128 = slice(128 * k, 128 * (k + 1))
        gsel = np.r_[128 * k:128 * (k + 1),
                     H + 128 * k:H + 128 * (k + 1),
                     2 * H + 128 * k:2 * H + 128 * (k + 1)]
        outWT_k = outW_pad[VS * k:VS * (k + 1)].T  # [H, VS]
        m = {
            "x32": x32,
            "h0": h0,
            "emb_sh": np.ascontiguousarray(emb_pad[ES * k:ES * (k + 1)]),
            "attnWT": np.ascontiguousarray(
                attnWT[:, sl64].reshape(128, 16, 64).reshape(128, 16 * 64)),
            "attnb": np.ascontiguousarray(attn_b[sl64]),
            "encT": np.ascontiguousarray(
                enc[:, sl128].reshape(128, 4, 128).reshape(128, 4 * 128)),
            "combWT": np.ascontiguousarray(
                combWT[:, sl128].reshape(128, 16, 128).reshape(128, 16 * 128)),
            "combb": np.ascontiguousarray(comb_b[sl128]),
            "WihT": np.ascontiguousarray(
                WihT[:, gsel].reshape(128, 8, 384).reshape(128, 8 * 384)),
            "WhhT": np.ascontiguousarray(
                WhhT[:, gsel].reshape(128, 8, 384).reshape(128, 8 * 384)),
            "bih": np.ascontiguousarray(b_ih[gsel]),
            "bhh": np.ascontiguousarray(b_hh[gsel]),
            "h0sl": np.ascontiguousarray(h0[sl128]),
            "outWT": np.ascontiguousarray(
                outWT_k.reshape(128, 8, VS).transpose(1, 0, 2)).astype(
                    ml_dtypes.bfloat16),
            "outb": np.ascontiguousarray(outb_pad[VS * k:VS * (k + 1)]),
        }
        in_maps.append(m)
    return in_maps


def kernel(**inputs):
    nc = _get_nc()
    in_maps = _prep_in_maps(**inputs)
    res = bass_utils.run_bass_kernel_spmd(nc, in_maps, core_ids=list(range(NCORES)))
    logp = np.concatenate([res.results[k]["out_z"] for k in range(NCORES)])[:V]
    attn = res.results[0]["out_attn"]
    return logp.reshape(1, V).astype(np.float32), attn.reshape(1, L).astype(np.float32)
